# revision 67
# baseline (speedup 1.0000x reference)
"""Trainium2 Bass kernel for GQA attention (B=2, S=2048, dim=2048, 32 Q / 8 KV heads, RoPE, causal).

Sharding (8 cores): data-parallel over batch (2) x tensor-parallel over heads (4 groups
of 8 Q heads / 2 KV heads). wq/wk/wv split column-wise, wo row-wise; per-core partial
outputs (fp16) are summed on the host (the row-parallel all-reduce).

Per-core device pipeline (all matmuls bf16 on the PE; warm-clock floor ~216ns per
512-column stream):
  xT [dim, s] (host-transposed)
  DMA is batched into few large transfers (each HWDGE trigger costs ~690ns of Sync
          issue time); the K-chain inputs lead so the PE starts ~4us in, and dummy
          matmuls fill the DMA-paced prologue waits so the PE HAM clock warms once
          and stays at full rate.
  Prologue runs the quarter-0 K/V/Q projections as per-output chains so each RoPE/
          transpose eviction pipelines behind the next chain.
  QT/KT = (w.T x.T) with RoPE fused into the PSUM eviction (rotate-half layout via
          host-side column permutation of wq/wk; scores are permutation-invariant).
          Q head tiles pair one kv0 head (partitions 0:64) with one kv1 head (64:128)
          so both PE array halves source K^T directly (row-packed K=64 matmul pairs,
          issued in 2-chunk batches so the half-array MMs overlap).
  VT -> V via PE transpose into vaug tiles carrying a ones column so the attention
          row-sums (softmax denominators) accumulate for free in PSUM row 64 of the
          attn@V output.
  scores^T[k, q] = K_h @ Q_h^T; exp on ScalarE straight out of PSUM (scale=1/8 fused);
          causal zeroing via affine_select on GpSimd only on diagonal blocks (those
          run early in the chunk order so the select latency hides).
  o^T/denom = [V|1]^T @ exp(scores^T); denominators broadcast across the partition
          halves via one K=64 PE matmul, reciprocal + scale on DVE.
  out = o^T.T @ wo accumulated over the 4 head pairs (wo rows host-permuted to match),
          evicted fp32->fp16 and DMA'd out; emitted as a generator pumped between
          attention chunks so the in-order PE queue never idles on exp pacing, with a
          [128,1024]-group tail variant once the score-PSUM ring frees up.
"""

import sys

for _p in ("/opt/trn_rl_repo", "/root/.axon_site/_ro/trn_rl_repo"):
    if _p not in sys.path:
        sys.path.append(_p)

from contextlib import ExitStack

import numpy as np

import concourse.bass as bass
import concourse.mybir as mybir
import concourse.tile as tile
from concourse import bacc
from concourse.bass_utils import run_bass_kernel_spmd


F32 = mybir.dt.float32
F32R = mybir.dt.float32r
F16 = mybir.dt.float16
F8 = mybir.dt.float8e4
MDT = mybir.dt.bfloat16  # matmul operand dtype (PSUM accumulation stays fp32)
import ml_dtypes
NP_MDT = ml_dtypes.bfloat16

HEAD_DIM = 64
N_HEADS = 32
N_KV_HEADS = 8
ROPE_THETA = 10000.0
B, S, DIM = 2, 2048, 2048
N_CORES = 8
NKC = DIM // 128  # 16 contraction chunks
NSQ = 4  # s quarters
SQW = S // NSQ  # 512
NPAIR = 4  # head pairs per core: pair j = (local head j [kv0], local head j+4 [kv1])
VAW = 130  # V_aug stride per k-chunk: 64 (kv0) + 1 (ones) + 64 (kv1) + 1 (ones)
USE_DMA_TRANSPOSE = False  # xbar 16-bit transpose for V; else PE-transpose path


def _rope_evict(nc, work, dst, psum, cs, sn, dst8=None):
    """PSUM -> SBUF eviction with rotate-half RoPE fused in.

    Tile layout on partitions: [head_a evens(32) | head_a odds(32) | head_b evens | head_b odds].
    The sin table carries the rotation signs baked per 32-partition block (-,+,-,+), so
    the partner-product lands partition-aligned and one full-tile add finishes the rotate.
    Two-SBUF-input ops must share a base partition; the swapped reads come from PSUM,
    which is exempt.

    With dst8 (an fp8 [64, 2, SQW] tile) the final add instead writes the
    DoubleRow score layout: head a (ki) on partitions 0:32 / head b on 32:64,
    with the even/odd 32-blocks folded into the ko free dimension.
    """
    ta = work.tile([128, SQW], F32, tag="ropeA")
    tb = work.tile([128, SQW], F32, tag="ropeB")
    nc.vector.tensor_mul(ta[:], psum[:], cs)
    nc.vector.tensor_mul(tb[0:32, :], psum[32:64, :], sn[0:32, :])
    nc.vector.tensor_mul(tb[32:64, :], psum[0:32, :], sn[32:64, :])
    nc.vector.tensor_mul(tb[64:96, :], psum[96:128, :], sn[64:96, :])
    nc.vector.tensor_mul(tb[96:128, :], psum[64:96, :], sn[96:128, :])
    if dst8 is None:
        nc.vector.tensor_add(dst[:], ta[:], tb[:])
    else:
        nc.vector.tensor_add(dst8[0:32, 0, :], ta[0:32, :], tb[0:32, :])
        nc.vector.tensor_add(dst8[0:32, 1, :], ta[32:64, :], tb[32:64, :])
        nc.vector.tensor_add(dst8[32:64, 0, :], ta[64:96, :], tb[64:96, :])
        nc.vector.tensor_add(dst8[32:64, 1, :], ta[96:128, :], tb[96:128, :])


def build_module():
    nc = bacc.Bacc(None, target_bir_lowering=False, debug=True)

    xT = nc.dram_tensor("xT", [DIM, S], MDT, kind="ExternalInput")
    wq = nc.dram_tensor("wq", [DIM, 512], MDT, kind="ExternalInput")
    wk = nc.dram_tensor("wk", [DIM, 128], MDT, kind="ExternalInput")
    wv = nc.dram_tensor("wv", [DIM, 128], MDT, kind="ExternalInput")
    wo = nc.dram_tensor("wo", [512, DIM], MDT, kind="ExternalInput")
    cos_d = nc.dram_tensor("cos", [128, S], F32, kind="ExternalInput")
    sin_d = nc.dram_tensor("sin", [128, S], F32, kind="ExternalInput")
    sel_d = nc.dram_tensor("sel", [64, 128], F32R, kind="ExternalInput")
    zsel_d = nc.dram_tensor("zsel", [64, 512], F32R, kind="ExternalInput")
    out_d = nc.dram_tensor("out", [S, DIM], F16, kind="ExternalOutput")

    with ExitStack() as ctx:
        tc = ctx.enter_context(tile.TileContext(nc))
        const = ctx.enter_context(tc.tile_pool(name="const", bufs=1))
        work = ctx.enter_context(tc.tile_pool(name="work", bufs=3))
        xq_pool = ctx.enter_context(tc.tile_pool(name="xq", bufs=2))
        cs_pool = ctx.enter_context(tc.tile_pool(name="cs", bufs=2))
        vt_pool = ctx.enter_context(tc.tile_pool(name="vt", bufs=2))
        qt_pool = ctx.enter_context(tc.tile_pool(name="qt", bufs=3))
        et_pool = ctx.enter_context(tc.tile_pool(name="et", bufs=6))
        ot_pool = ctx.enter_context(tc.tile_pool(name="ot", bufs=3))
        st_pool = ctx.enter_context(tc.tile_pool(name="st", bufs=4))
        dn_pool = ctx.enter_context(tc.tile_pool(name="dn", bufs=2))
        ps_qw = ctx.enter_context(tc.tile_pool(name="ps_qw", bufs=2, space="PSUM"))
        ps_s = ctx.enter_context(tc.tile_pool(name="ps_s", bufs=2, space="PSUM"))
        ps_o = ctx.enter_context(tc.tile_pool(name="ps_o", bufs=1, space="PSUM"))

        # ---- persistent SBUF tensors ----
        wq_sb = const.tile([128, NKC, 512], MDT)
        wk_sb = const.tile([128, NKC, 128], MDT)
        wv_sb = const.tile([128, NKC, 128], MDT)
        wo_sb = const.tile([128, NPAIR, DIM], MDT)
        # K^T quarters (rope'd; partitions = d of [kv0|kv1]) and V chunks (+ones cols),
        # split per quarter/chunk to keep producer/consumer dependency ranges tight
        kt_sb = [const.tile([128, SQW], MDT, name=f"kt{i}") for i in range(NSQ)]
        vaug_sb = [const.tile([128, VAW], MDT, name=f"vaug{i}") for i in range(NKC)]
        sel2 = const.tile([64, 128], F32R)
        recsel = const.tile([64, SQW], F32R)

        wq_r = wq.rearrange("(kc p) m -> p kc m", p=128)
        wk_r = wk.rearrange("(kc p) m -> p kc m", p=128)
        wv_r = wv.rearrange("(kc p) m -> p kc m", p=128)
        wo_r = wo.rearrange("(j p) n -> p j n", p=128)

        # broadcast-selector for the per-head denominator rows (K=64 matmul);
        # recsel rows other than 0/32 stay zero so they contribute nothing
        nc.sync.dma_start(out=sel2[:], in_=sel_d[:])
        nc.sync.dma_start(out=recsel[:], in_=zsel_d[:])
        for kc in range(NKC):
            nc.vector.memset(vaug_sb[kc][:], 1.0)

        # Each HWDGE trigger costs ~690ns of Sync-sequencer issue time, so DMAs
        # are batched: the whole xq quarter is ONE pool tile written by 4
        # four-chunk transfers (region-level dependency tracking keeps the
        # per-chunk consumer pacing), and each weight tensor is 1-2 transfers.
        xT_r = xT.rearrange("(c p) s -> p c s", p=128)

        def dma_quarter(sq):
            cs = cs_pool.tile([128, SQW], F32, tag="cos", name="cs")
            sn = cs_pool.tile([128, SQW], F32, tag="sin", name="sn")
            nc.sync.dma_start(out=cs[:], in_=cos_d[:, sq * SQW : (sq + 1) * SQW])
            nc.sync.dma_start(out=sn[:], in_=sin_d[:, sq * SQW : (sq + 1) * SQW])
            xqt = xq_pool.tile([128, NKC, SQW], MDT, tag="xq", name="xq")
            for g in range(4):
                nc.sync.dma_start(
                    out=xqt[:, 4 * g : 4 * g + 4, :],
                    in_=xT_r[:, 4 * g : 4 * g + 4, sq * SQW : (sq + 1) * SQW],
                )
            xq = [xqt[:, kc, :] for kc in range(NKC)]
            return cs, sn, xq

        # ---- prologue DMA: quarter-0 activations and all weights; the first
        # K-chain inputs (xq group 0 + wk half 0) lead so the PE starts ~4us in;
        # cos/sin follow (first needed at the kt rope ~15us in) ----
        cs0 = cs_pool.tile([128, SQW], F32, tag="cos", name="cs")
        sn0 = cs_pool.tile([128, SQW], F32, tag="sin", name="sn")
        xqt0 = xq_pool.tile([128, NKC, SQW], MDT, tag="xq", name="xq")
        nc.sync.dma_start(out=xqt0[:, 0:2, :], in_=xT_r[:, 0:2, 0:SQW])
        nc.sync.dma_start(out=wk_sb[:, 0:4, :], in_=wk_r[:, 0:4, :])
        nc.sync.dma_start(out=xqt0[:, 2:4, :], in_=xT_r[:, 2:4, 0:SQW])
        nc.sync.dma_start(out=wk_sb[:, 4:8, :], in_=wk_r[:, 4:8, :])
        nc.sync.dma_start(out=xqt0[:, 4:8, :], in_=xT_r[:, 4:8, 0:SQW])
        nc.sync.dma_start(out=wk_sb[:, 8:16, :], in_=wk_r[:, 8:16, :])
        for g in range(2, 4):
            nc.sync.dma_start(
                out=xqt0[:, 4 * g : 4 * g + 4, :],
                in_=xT_r[:, 4 * g : 4 * g + 4, 0:SQW],
            )
        nc.sync.dma_start(out=wv_sb[:], in_=wv_r[:])
        nc.sync.dma_start(out=wq_sb[:, 0:8, :], in_=wq_r[:, 0:8, :])
        nc.sync.dma_start(out=cs0[:], in_=cos_d[:, 0:SQW])
        nc.sync.dma_start(out=sn0[:], in_=sin_d[:, 0:SQW])
        nc.sync.dma_start(out=wq_sb[:, 8:16, :], in_=wq_r[:, 8:16, :])
        pre = {0: (cs0, sn0, [xqt0[:, kc, :] for kc in range(NKC)])}

        VT_DT = MDT if USE_DMA_TRANSPOSE else F32
        if not USE_DMA_TRANSPOSE:
            from concourse.masks import make_identity
            ident = const.tile([128, 128], F32)
            make_identity(nc, ident[:])

        def vtrans(sq, vtq):
            """V^T quarter -> per-chunk [V|1] tiles."""
            for t in range(4):
                kc = sq * 4 + t
                csl = slice(t * 128, (t + 1) * 128)
                if USE_DMA_TRANSPOSE:
                    # xbar needs 16B-aligned targets: transpose into an aligned
                    # staging tile, then one strided copy interleaves the ones cols
                    stg = vt_pool.tile([128, 128], MDT, tag="stg", name=f"stg{kc}")
                    nc.sync.dma_start_transpose(out=stg[:, 0:64], in_=vtq[0:64, csl])
                    nc.sync.dma_start_transpose(out=stg[:, 64:128], in_=vtq[64:128, csl])
                    dst = vaug_sb[kc][:, 0:130].rearrange("p (h m) -> p h m", h=2)[:, :, 0:64]
                    src = stg[:].rearrange("p (h m) -> p h m", h=2)
                    nc.scalar.copy(dst, src)
                else:
                    pt = ps_o.tile([128, SQW], F32, tag="oA", name=f"pt{kc}")
                    nc.tensor.transpose(pt[:, 0:128], vtq[:, csl], ident[:])
                    nc.scalar.copy(vaug_sb[kc][:, 0:64], pt[:, 0:64])
                    nc.scalar.copy(vaug_sb[kc][:, 65:129], pt[:, 64:128])

        def kv_mms(sq):
            _, _, xq = pre[sq]
            ps_k = ps_qw.tile([128, SQW], F32, tag="qw", name="ps_k")
            ps_v = ps_qw.tile([128, SQW], F32, tag="qw", name="ps_v")
            for kc in range(NKC):
                nc.tensor.matmul(
                    ps_k[:], wk_sb[:, kc, :], xq[kc][:],
                    start=(kc == 0), stop=(kc == NKC - 1),
                )
                nc.tensor.matmul(
                    ps_v[:], wv_sb[:, kc, :], xq[kc][:],
                    start=(kc == 0), stop=(kc == NKC - 1),
                )
            return ps_k, ps_v

        def kv_evict(sq, ps_k, ps_v):
            cs, sn, _ = pre[sq]
            _rope_evict(nc, work, kt_sb[sq][:], ps_k, cs[:], sn[:])
            vtq = vt_pool.tile([128, SQW], VT_DT, tag="vt")
            nc.scalar.copy(vtq[:], ps_v[:])
            return vtq

        def qproj_mms(sq, jh):
            _, _, xq = pre[sq]
            pss = [ps_qw.tile([128, SQW], F32, tag="qw", name=f"psq{jh}_{i}") for i in range(2)]
            for kc in range(NKC):
                for jj in range(2):
                    j = jh * 2 + jj
                    nc.tensor.matmul(
                        pss[jj][:],
                        wq_sb[:, kc, j * 128 : (j + 1) * 128],
                        xq[kc][:],
                        start=(kc == 0), stop=(kc == NKC - 1),
                    )
            return pss

        def qproj_evict(sq, jh, pss):
            cs, sn, _ = pre[sq]
            out = []
            for jj in range(2):
                j = jh * 2 + jj
                t = qt_pool.tile([128, SQW], MDT, tag=f"qt{j}", name=f"qt{j}")
                _rope_evict(nc, work, t[:], pss[jj], cs[:], sn[:])
                out.append(t)
            return out

        def prologue():
            """Quarter-0 K/V/Q projections as per-output chains (K first) so the
            RoPE/transpose evictions pipeline behind the later chains instead of
            serializing after all projections. PSUM: kv on the qw ring, Q tiles
            packed into the score-pool [128,1024] tiles (free this early).

            The K-chain is paced by the xq DMA; dummy matmuls on scratch data
            fill the waits so the PE HAM clock warms once (~3.4us of sustained
            activity) instead of running the whole prologue at K=4/8 half-rate."""
            dumw = const.tile([128, 128], MDT, name="dumw")
            dumx = const.tile([128, SQW], MDT, name="dumx")
            nc.vector.memset(dumw[:], 1.0)
            nc.vector.memset(dumx[:], 1.0)
            dps = ps_o.tile([128, SQW], F32, tag="oB", name="dummy")

            def dummy(n):
                for _ in range(n):
                    nc.tensor.matmul(dps[:], dumw[:], dumx[:], start=True, stop=True)

            cs, sn, xq = pre[0]
            ps_k = ps_qw.tile([128, SQW], F32, tag="qw", name="ps_k")
            ps_v = ps_qw.tile([128, SQW], F32, tag="qw", name="ps_v")
            pq01 = ps_s.tile([128, 2 * SQW], F32, tag="s", name="pq01")
            pq23 = ps_s.tile([128, 2 * SQW], F32, tag="s", name="pq23")
            qslices = [
                pq01[:, 0:SQW], pq01[:, SQW : 2 * SQW],
                pq23[:, 0:SQW], pq23[:, SQW : 2 * SQW],
            ]
            dummy(6)
            for kc in range(NKC):
                nc.tensor.matmul(ps_k[:], wk_sb[:, kc, :], xq[kc][:],
                                 start=(kc == 0), stop=(kc == NKC - 1))
                dummy(2)
            _rope_evict(nc, work, kt_sb[0][:], ps_k, cs[:], sn[:])
            for kc in range(NKC):
                nc.tensor.matmul(ps_v[:], wv_sb[:, kc, :], xq[kc][:],
                                 start=(kc == 0), stop=(kc == NKC - 1))
            vtq = vt_pool.tile([128, SQW], VT_DT, tag="vt")
            nc.scalar.copy(vtq[:], ps_v[:])
            vtrans(0, vtq)
            qt = []
            for j in range(4):
                for kc in range(NKC):
                    nc.tensor.matmul(
                        qslices[j], wq_sb[:, kc, j * 128 : (j + 1) * 128], xq[kc][:],
                        start=(kc == 0), stop=(kc == NKC - 1),
                    )
                t = qt_pool.tile([128, SQW], MDT, tag=f"qt{j}", name=f"qt{j}")
                _rope_evict(nc, work, t[:], qslices[j], cs[:], sn[:])
                qt.append(t)
            return qt

        def pump(flr, n=1):
            if flr is not None:
                for _ in range(n):
                    if next(flr, None) is None:
                        return

        def attn_pair(sq, j, qtj, flr):
            n_kc = 4 * (sq + 1)  # causal: this q-quarter sees k chunks 0..n_kc-1
            # diagonal chunks early (their exp -> affine_select latency hides
            # behind the affine-free off-diagonal tail; attnV accumulation is
            # order-independent) but not first (they need the freshest kt
            # quarter, whose rope may still be in flight at stream start)
            off = list(range(0, 4 * sq))
            kcs = off[:2] + list(range(4 * sq, n_kc)) + off[2:]
            po_a = ps_o.tile([128, SQW], F32, tag="oA", name=f"poa{j}")
            po_b = ps_o.tile([128, SQW], F32, tag="oB", name=f"pob{j}")
            def attnv(pos, kc, et):
                nc.tensor.matmul(
                    po_a[0:65, :], vaug_sb[kc][:, 0:65], et[:, 0:512],
                    start=(pos == 0), stop=(pos == n_kc - 1),
                )
                nc.tensor.matmul(
                    po_b[0:65, :], vaug_sb[kc][:, 65:130], et[:, 512:1024],
                    start=(pos == 0), stop=(pos == n_kc - 1),
                )

            def scores(kc):
                # row-packed K=64 pair: the two matmuls land on disjoint PE row
                # halves and (partially) overlap
                pss = ps_s.tile([128, 2 * SQW], F32, tag="s", name=f"pss{j}_{kc}")
                et = et_pool.tile([128, 2 * SQW], MDT, tag="et", name=f"et{j}_{kc}")
                ksl = slice((kc % 4) * 128, (kc % 4 + 1) * 128)
                nc.tensor.matmul(
                    pss[:, 0:512], kt_sb[kc // 4][0:64, ksl], qtj[0:64, :],
                    start=True, stop=True,
                )
                nc.tensor.matmul(
                    pss[:, 512:1024], kt_sb[kc // 4][64:128, ksl], qtj[64:128, :],
                    start=True, stop=True,
                )
                nc.scalar.activation(
                    et[:], pss[:], mybir.ActivationFunctionType.Exp,
                    scale=float(1.0 / np.sqrt(HEAD_DIM)),
                )
                if kc >= 4 * sq:  # diagonal block: zero the q < k region on GpSimd
                    # keep et[p, h*512+q] where (sq*512+q) >= (kc*128+p)
                    nc.gpsimd.affine_select(
                        out=et[:], in_=et[:],
                        compare_op=mybir.AluOpType.is_ge,
                        fill=0.0,
                        base=sq * 512 - kc * 128,
                        pattern=[[0, 2], [1, 512]],
                        channel_multiplier=-1,
                    )
                return et

            # two-kc batches: 4 alternating K=64 half-array score MMs pipeline
            # back-to-back, then the (lagged) attnV pairs follow
            prev = []
            for b0 in range(0, n_kc, 2):
                ets = [(b0, kcs[b0], scores(kcs[b0])),
                       (b0 + 1, kcs[b0 + 1], scores(kcs[b0 + 1]))]
                for p in prev:
                    attnv(*p)
                prev = ets
                pump(flr, 2 if (sq < NSQ - 1 or j >= 2) else 1)
            for p in prev:
                attnv(*p)
            return po_a, po_b

        def norm_pair(sq, j, po_a, po_b):
            ot = ot_pool.tile([128, SQW], MDT, tag=f"ot{j}", name=f"ot{j}")
            nc.vector.tensor_copy(ot[0:64, :], po_a[0:64, :])
            nc.vector.tensor_copy(ot[64:128, :], po_b[0:64, :])
            # softmax denominators: broadcast the raw row-sums across the partition
            # halves via one K=64 matmul, then one full-partition recip
            # ScalarE is exp-saturated in the final quarter: keep the denominator
            # copies off its queue there so prb doesn't stall
            den_copy = nc.vector.tensor_copy if sq == NSQ - 1 else nc.scalar.copy
            den_copy(recsel[0:1, :], po_a[64:65, :])
            den_copy(recsel[32:33, :], po_b[64:65, :])
            prb = ps_o.tile([128, SQW], F32, tag="oA", name=f"prb{j}")
            nc.tensor.matmul(prb[:], sel2[:], recsel[:], start=True, stop=True)
            with nc.allow_low_precision(reason="approx reciprocal of softmax denominators"):
                recd = work.tile([128, SQW], F32, tag="recd")
                nc.vector.tensor_copy(recd[:], prb[:])
                rec = work.tile([128, SQW], F32, tag="rec")
                nc.vector.reciprocal_approx_fast(rec[:], recd[:])
            nc.vector.tensor_mul(ot[:], ot[:], rec[:])
            return ot

        def wo_gen(sq, ots):
            """Output-projection for quarter sq as a generator: yields after each
            PE matmul / eviction so the emission can be pumped between the next
            quarter's attention quads (filler for the in-order PE queue)."""
            for qc in range(4):
                for half in range(2):
                    st = st_pool.tile([128, 2 * SQW], F16, tag="st", name=f"st{qc}_{half}")
                    for i in range(2):
                        n = 2 * half + i
                        pw = ps_qw.tile([128, SQW], F32, tag="qw", name=f"pw{qc}_{n}")
                        for j in range(NPAIR):
                            nc.tensor.matmul(
                                pw[:],
                                ots[j][:, qc * 128 : (qc + 1) * 128],
                                wo_sb[:, j, n * 512 : (n + 1) * 512],
                                start=(j == 0), stop=(j == NPAIR - 1),
                            )
                            yield True
                        dst = st[:, i * 512 : (i + 1) * 512]
                        # wo(2) runs interleaved into quarter 3 where ScalarE is
                        # saturated by exp: keep its evictions off Scalar
                        if sq == NSQ - 2 or n % 2 == 0:
                            nc.vector.tensor_copy(dst, pw[:])
                        else:
                            nc.scalar.copy(dst, pw[:])
                        yield True
                    nc.sync.dma_start(
                        out=out_d[(sq * 4 + qc) * 128 : (sq * 4 + qc + 1) * 128,
                                  half * 1024 : (half + 1) * 1024],
                        in_=st[:],
                    )

        def wo_gen2(sq, ots):
            """Tail variant of wo_gen: the score-PSUM ring is free once the last
            stream ends, so accumulate n-pairs into [128,1024] tiles with one
            (alternating-engine) eviction each."""
            for qc in range(4):
                for half in range(2):
                    pw2 = ps_s.tile([128, 2 * SQW], F32, tag="s", name=f"pw2_{qc}_{half}")
                    st = st_pool.tile([128, 2 * SQW], F16, tag="st", name=f"st2{qc}_{half}")
                    for i in range(2):
                        n = 2 * half + i
                        for j in range(NPAIR):
                            nc.tensor.matmul(
                                pw2[:, i * 512 : (i + 1) * 512],
                                ots[j][:, qc * 128 : (qc + 1) * 128],
                                wo_sb[:, j, n * 512 : (n + 1) * 512],
                                start=(j == 0), stop=(j == NPAIR - 1),
                            )
                            yield True
                    if half == 0:
                        nc.vector.tensor_copy(st[:], pw2[:])
                    else:
                        nc.scalar.copy(st[:], pw2[:])
                    yield True
                    nc.sync.dma_start(
                        out=out_d[(sq * 4 + qc) * 128 : (sq * 4 + qc + 1) * 128,
                                  half * 1024 : (half + 1) * 1024],
                        in_=st[:],
                    )

        # software-pipelined quarters: the NEXT quarter's projections and the
        # PREVIOUS quarter's output projection are interleaved into this quarter's
        # attention streams so the (in-order) PE queue always has matmul work
        # while the streams wait on ScalarE exp / the normalization chain
        qt_cur = prologue()
        pre[1] = dma_quarter(1)
        for h in range(2):  # wo last: first needed mid-quarter-1
            nc.sync.dma_start(
                out=wo_sb[:, 2 * h : 2 * h + 2, :], in_=wo_r[:, 2 * h : 2 * h + 2, :]
            )
        pend_wo = None
        for sq in range(NSQ):
            flr = wo_gen(sq - 1, pend_wo) if pend_wo is not None else None
            last = sq == NSQ - 1
            ots = []
            qt_next = []
            for jp in range(4):
                poab = attn_pair(sq, jp, qt_cur[jp], flr)
                # post-attn PE filler covering the denominator-copy latency
                # before the prb matmul: next-quarter projection MMs (their
                # PSUM evictions are deferred past the norm; no wo pumps may
                # interleave while those accumulators are live)
                if not last:
                    if jp == 0:
                        kvps = kv_mms(sq + 1)
                    elif jp == 1:
                        pump(flr, 4)
                    elif jp == 2:
                        q01ps = qproj_mms(sq + 1, 0)
                    else:
                        q23ps = qproj_mms(sq + 1, 1)
                else:
                    pump(flr, 4)
                ots.append(norm_pair(sq, jp, *poab))
                if not last:
                    if jp == 0:
                        vtq = kv_evict(sq + 1, *kvps)
                    elif jp == 1:
                        vtrans(sq + 1, vtq)
                        if sq + 2 < NSQ:
                            pre[sq + 2] = dma_quarter(sq + 2)
                    elif jp == 2:
                        qt_next += qproj_evict(sq + 1, 0, q01ps)
                    else:
                        qt_next += qproj_evict(sq + 1, 1, q23ps)
            pump(flr, 200)  # drain the previous quarter's leftover wo work
            pend_wo = ots
            qt_cur = qt_next
        for _ in wo_gen2(NSQ - 1, pend_wo):
            pass

    nc.compile()
    return nc


_NC_CACHE = {}


def _get_module():
    if "nc" not in _NC_CACHE:
        _NC_CACHE["nc"] = build_module()
    return _NC_CACHE["nc"]


def _rope_tables():
    inv = 1.0 / (ROPE_THETA ** (np.arange(0, HEAD_DIM, 2, dtype=np.float32) / HEAD_DIM))
    ang = np.arange(S, dtype=np.float32)[:, None] * inv[None, :].astype(np.float32)  # [S, 32]
    cos = np.cos(ang).T.astype(np.float32)  # [32, S]
    sin = np.sin(ang).T.astype(np.float32)
    # sin rows carry the rotate-half signs per 32-partition block: [-, +, -, +]
    sin_s = np.concatenate([-sin, sin, -sin, sin], axis=0)
    return (
        np.ascontiguousarray(np.tile(cos, (4, 1))),
        np.ascontiguousarray(sin_s),
    )  # [128, S]


def make_in_maps(x, wq, wk, wv, wo):
    perm64 = np.concatenate([np.arange(0, 64, 2), np.arange(1, 64, 2)])
    cos_t, sin_t = _rope_tables()
    sel_t = np.zeros((64, 128), dtype=np.float32)
    sel_t[0, 0:64] = 1.0
    sel_t[32, 64:128] = 1.0
    xts = [np.ascontiguousarray(x[b].T).astype(NP_MDT) for b in range(B)]
    in_maps = []
    for c in range(N_CORES):
        b, g = divmod(c, 4)
        # Q tile j pairs local head j (kv0) with local head j+4 (kv1); rotate-half perm
        qidx = np.concatenate(
            [np.concatenate([512 * g + 64 * j + perm64, 512 * g + 64 * (j + 4) + perm64])
             for j in range(4)]
        )
        kidx = np.concatenate([128 * g + 64 * kvl + perm64 for kvl in range(2)])
        widx = np.concatenate(
            [np.concatenate([512 * g + 64 * j + np.arange(64),
                             512 * g + 64 * (j + 4) + np.arange(64)])
             for j in range(4)]
        )
        in_maps.append({
            "xT": xts[b],
            "wq": np.ascontiguousarray(wq[:, qidx]).astype(NP_MDT),
            "wk": np.ascontiguousarray(wk[:, kidx]).astype(NP_MDT),
            "wv": np.ascontiguousarray(wv[:, 128 * g : 128 * (g + 1)]).astype(NP_MDT),
            "wo": np.ascontiguousarray(wo[widx, :]).astype(NP_MDT),
            "cos": cos_t,
            "sin": sin_t,
            "sel": sel_t,
            "zsel": np.zeros((64, 512), dtype=np.float32),
        })
    return in_maps


def run(in_maps, **kwargs):
    nc = _get_module()
    return run_bass_kernel_spmd(nc, in_maps, core_ids=list(range(N_CORES)), **kwargs)


def kernel(x, wq, wk, wv, wo):
    x = np.asarray(x, dtype=np.float32)
    wq = np.asarray(wq, dtype=np.float32)
    wk = np.asarray(wk, dtype=np.float32)
    wv = np.asarray(wv, dtype=np.float32)
    wo = np.asarray(wo, dtype=np.float32)
    res = run(make_in_maps(x, wq, wk, wv, wo))
    out = np.zeros((B, S, DIM), dtype=np.float32)
    for c in range(N_CORES):
        out[c // 4] += res.results[c]["out"].astype(np.float32)
    return out
